# revision 13
# baseline (speedup 1.0000x reference)
"""Trainium2 Bass kernel for nn_Decoder_4561255269164 (retrieval_knn).

Math: the reference's top-K(8) KNN collapses to min-reductions:
  - backward: weight w=1/sqrt(d) is nonzero only where d equals the row min
    (over kept candidates), so the scatter-add num/den equals
    E_b^T @ [w*rgb, w] with E_b[i,j] = (d2[i,j] == rowmin_i).
  - forward: only the column argmin rows of d2 matter; sumf/cntf =
    E_f^T @ [rgb, 1] with E_f[i,j] = (d2[i,j] <= colmin_j * (1+1e-6)).
  - exact-match (d==0) rows use a separate weight column gated by rowmin==0.

Sharding: targets (N) split across cores (padded to NT*128 rows each).
Each core computes rowmins for its shard, partial colmins (AllReduce-min),
then indicator matmuls accumulating [12, L] partials (AllReduce-add), and a
redundant O(L) finalize. d2 is computed on the fly by K=5 augmented matmuls
(never stored): s[i,j] = -2*t_i.c_j + a2_i + b2_j, d2 = relu(s); the keep
mask folds into b2 (unkept -> 1e30).
"""

import numpy as np

import concourse.bass as bass
import concourse.bacc as bacc
import concourse.mybir as mybir
import concourse.tile as tile
from concourse import library_config
from concourse.bass_utils import run_bass_kernel_spmd

F32 = mybir.dt.float32
AX = mybir.AxisListType
ALU = mybir.AluOpType
ACTF = mybir.ActivationFunctionType

# geometry (overridable for small-scale simulation tests)
NCORES = 8
L = 16384          # candidates
N = 10000          # targets
NT = 10            # i-tiles of 128 per core (pad 1250 -> 1280)
POINTS_NUM = 8192
BIG = np.float32(1e30)
FWD_EPS = 1.000001  # relative margin for forward colmin match


def _build_nc():
    nsh = N // NCORES
    npad = NT * 128
    at_w = min(2048, L)           # pass A-T chunk width
    b_w = min(1024, L)            # pass B chunk width
    nq = b_w // 512 if b_w >= 512 else 1

    nc = bacc.Bacc("TRN2", target_bir_lowering=False, debug=False,
                   num_devices=NCORES)

    c5r = nc.declare_dram_parameter("c5r", [5, L], F32, isOutput=False)
    c5m = nc.declare_dram_parameter("c5m", [5, L], F32, isOutput=False)
    t5d = nc.declare_dram_parameter("t5", [5, npad], F32, isOutput=False)
    trgbd = nc.declare_dram_parameter("trgb", [128, NT * 3], F32, isOutput=False)
    rgbpd = nc.declare_dram_parameter("rgbp", [3, L], F32, isOutput=False)
    keepd = nc.declare_dram_parameter("keepf", [1, L], F32, isOutput=False)
    predd = nc.declare_dram_parameter("predf", [1, L], F32, isOutput=False)
    ktgtd = nc.declare_dram_parameter("ktgt", [1, L], F32, isOutput=False)
    eyed = nc.declare_dram_parameter("eye128", [128, 128], F32, isOutput=False)
    outd = nc.declare_dram_parameter("out", [1, 2], F32, isOutput=True)

    rg = [list(range(NCORES))]

    with tile.TileContext(nc) as tc:
        nc.gpsimd.load_library(library_config.mlp)
        with (
            tc.tile_pool(name="persist", bufs=1) as pp,
            tc.tile_pool(name="dram", bufs=1, space="DRAM") as dp,
        ):
            t5s = pp.tile([5, npad], F32, tag="t5s")
            nc.sync.dma_start(t5s[:], t5d[:, :])
            trgb = pp.tile([128, NT * 3], F32, tag="trgb")
            nc.sync.dma_start(trgb[:], trgbd[:, :])
            eye = pp.tile([128, 128], F32, tag="eye")
            nc.sync.dma_start(eye[:], eyed[:, :])

            m_all = pp.tile([128, NT], F32, tag="m_all")   # raw row mins
            m_relu = pp.tile([128, NT], F32, tag="m_relu")
            wb_all = pp.tile([128, NT * 8], F32, tag="wb_all")
            wf_all = pp.tile([128, NT * 4], F32, tag="wf_all")
            m2loc = pp.tile([128, L // 128], F32, tag="m2loc")  # [p, jt]

            m2_in = dp.tile([L // 128, 128], F32, tag="m2_in")   # j-linear
            m2_out = dp.tile([1, L], F32, tag="m2_out")
            nd_in = dp.tile([12, L], F32, tag="nd_in")
            nd_out = dp.tile([12, L], F32, tag="nd_out")

            # ---------------- Pass A-T: row mins over kept (masked) ----------
            with (
                tc.tile_pool(name="at_cm", bufs=3) as cmp_,
                tc.tile_pool(name="at_ps", bufs=2, space="PSUM") as psp,
                tc.tile_pool(name="at_r", bufs=2) as rp,
            ):
                nat = L // at_w
                for t in range(NT):
                    rmin = rp.tile([128, nat], F32, tag="rmin")
                    for jc in range(nat):
                        cm = cmp_.tile([5, at_w], F32, tag="cm")
                        nc.sync.dma_start(cm[:], c5m[:, jc * at_w:(jc + 1) * at_w])
                        ps = psp.tile([128, at_w], F32, tag="ps")
                        for q0 in range(0, at_w, 512):
                            qw = min(512, at_w - q0)
                            nc.tensor.matmul(
                                ps[:, q0:q0 + qw],
                                lhsT=t5s[:, t * 128:(t + 1) * 128],
                                rhs=cm[:, q0:q0 + qw],
                                start=True, stop=True)
                        nc.vector.tensor_reduce(
                            rmin[:, jc:jc + 1], ps[:], axis=AX.X, op=ALU.min)
                    nc.vector.tensor_reduce(
                        m_all[:, t:t + 1], rmin[:], axis=AX.X, op=ALU.min)

            # ---------------- Pass A-C: local col mins (all targets) ---------
            with (
                tc.tile_pool(name="ac_c", bufs=3) as cp2,
                tc.tile_pool(name="ac_ps", bufs=2, space="PSUM") as psp2,
            ):
                for jt in range(L // 128):
                    c5 = cp2.tile([5, 128], F32, tag="c5")
                    nc.sync.dma_start(c5[:], c5r[:, jt * 128:(jt + 1) * 128])
                    ps = psp2.tile([128, npad], F32, tag="ps2")
                    for q0 in range(0, npad, 512):
                        qw = min(512, npad - q0)
                        nc.tensor.matmul(ps[:, q0:q0 + qw], lhsT=c5[:],
                                         rhs=t5s[:, q0:q0 + qw],
                                         start=True, stop=True)
                    nc.vector.tensor_reduce(
                        m2loc[:, jt:jt + 1], ps[:], axis=AX.X, op=ALU.min)

            # transpose m2loc -> [jt, p] so DRAM layout is j-linear, then
            # relu + margin-scale and AllReduce(min).
            with (
                tc.tile_pool(name="tr_ps", bufs=1, space="PSUM") as trp,
                tc.tile_pool(name="tr_sb", bufs=1) as trs,
            ):
                pst = trp.tile([128, 128], F32, tag="pst")
                nc.tensor.transpose(pst[0:L // 128, :], m2loc[:], eye[:])
                m2t = trs.tile([L // 128, 128], F32, tag="m2t")
                nc.vector.tensor_scalar(m2t[:], pst[0:L // 128, :], 0.0,
                                        float(FWD_EPS),
                                        op0=ALU.max, op1=ALU.mult)
                nc.sync.dma_start(m2_in[:, :], m2t[:])
            nc.gpsimd.collective_compute(
                "AllReduce", ALU.min, replica_groups=rg,
                ins=[m2_in.opt()], outs=[m2_out.opt()])

            # ---------------- weight tiles ----------------------------------
            with tc.tile_pool(name="wsmall", bufs=1) as ws:
                nc.vector.tensor_scalar(m_relu[:], m_all[:], 0.0, None,
                                        op0=ALU.max)
                msafe = ws.tile([128, NT], F32, tag="msafe")
                nc.vector.tensor_scalar(msafe[:], m_relu[:], 1e-30, None,
                                        op0=ALU.max)
                sqm = ws.tile([128, NT], F32, tag="sqm")
                nc.scalar.activation(sqm[:], msafe[:], ACTF.Sqrt)
                w0 = ws.tile([128, NT], F32, tag="w0")
                nc.vector.reciprocal(w0[:], sqm[:])
                vv = ws.tile([128, NT], F32, tag="vv")
                nc.vector.tensor_scalar(vv[:], m_relu[:], 0.0, None, op0=ALU.is_gt)
                v2 = ws.tile([128, NT], F32, tag="v2")
                nc.vector.tensor_scalar(v2[:], m_relu[:], 1e29, None, op0=ALU.is_lt)
                nc.vector.tensor_tensor(vv[:], vv[:], v2[:], op=ALU.mult)
                wgt = ws.tile([128, NT], F32, tag="wgt")
                nc.vector.tensor_tensor(wgt[:], w0[:], vv[:], op=ALU.mult)
                zz = ws.tile([128, NT], F32, tag="zz")
                nc.vector.tensor_scalar(zz[:], m_relu[:], 0.0, None,
                                        op0=ALU.is_equal)

                for t in range(NT):
                    for c in range(3):
                        nc.vector.tensor_tensor(
                            wb_all[:, t * 8 + c:t * 8 + c + 1],
                            wgt[:, t:t + 1], trgb[:, t * 3 + c:t * 3 + c + 1],
                            op=ALU.mult)
                        nc.vector.tensor_tensor(
                            wb_all[:, t * 8 + 4 + c:t * 8 + 5 + c],
                            zz[:, t:t + 1], trgb[:, t * 3 + c:t * 3 + c + 1],
                            op=ALU.mult)
                        nc.vector.tensor_copy(
                            wf_all[:, t * 4 + c:t * 4 + c + 1],
                            trgb[:, t * 3 + c:t * 3 + c + 1])
                    nc.vector.tensor_copy(wb_all[:, t * 8 + 3:t * 8 + 4],
                                          wgt[:, t:t + 1])
                    nc.vector.tensor_copy(wb_all[:, t * 8 + 7:t * 8 + 8],
                                          zz[:, t:t + 1])
                    nc.vector.memset(wf_all[:, t * 4 + 3:t * 4 + 4], 1.0)

            # ---------------- Pass B: indicators + scatter matmuls ----------
            with (
                tc.tile_pool(name="b_c", bufs=2) as bcp,
                tc.tile_pool(name="b_m2r", bufs=2) as bm2,
                tc.tile_pool(name="b_m2b", bufs=2) as bm2b,
                tc.tile_pool(name="b_d2", bufs=3) as bd2,
                tc.tile_pool(name="b_e", bufs=3) as bep,
                tc.tile_pool(name="b_psd", bufs=2, space="PSUM") as bpsd,
                tc.tile_pool(name="b_acc", bufs=1, space="PSUM") as baccp,
            ):
                for jc in range(L // b_w):
                    c5c = bcp.tile([5, b_w], F32, tag="c5c")
                    nc.sync.dma_start(c5c[:], c5r[:, jc * b_w:(jc + 1) * b_w])
                    m2rw = bm2.tile([1, b_w], F32, tag="m2rw")
                    nc.sync.dma_start(m2rw[:],
                                      m2_out[:, jc * b_w:(jc + 1) * b_w])
                    m2b = bm2b.tile([128, b_w], F32, tag="m2b")
                    nc.gpsimd.partition_broadcast(m2b[:], m2rw[:])

                    accb = [baccp.tile([8, 512], F32, tag=f"accb{q}",
                                       name=f"accb{q}") for q in range(nq)]
                    accf = [baccp.tile([4, 512], F32, tag=f"accf{q}",
                                       name=f"accf{q}") for q in range(nq)]
                    for t in range(NT):
                        psd = bpsd.tile([128, b_w], F32, tag="psd")
                        for q0 in range(0, b_w, 512):
                            qw = min(512, b_w - q0)
                            nc.tensor.matmul(
                                psd[:, q0:q0 + qw],
                                lhsT=t5s[:, t * 128:(t + 1) * 128],
                                rhs=c5c[:, q0:q0 + qw],
                                start=True, stop=True)
                        d2s = bd2.tile([128, b_w], F32, tag="d2s")
                        nc.scalar.activation(d2s[:], psd[:], ACTF.Relu)
                        eb = bep.tile([128, b_w], F32, tag="eb")
                        nc.vector.tensor_scalar(eb[:], d2s[:],
                                                m_relu[:, t:t + 1],
                                                None, op0=ALU.is_equal)
                        ef = bep.tile([128, b_w], F32, tag="ef")
                        nc.vector.tensor_tensor(ef[:], d2s[:], m2b[:],
                                                op=ALU.is_le)
                        for q in range(nq):
                            qw = min(512, b_w - q * 512)
                            nc.tensor.matmul(
                                accb[q][:, 0:qw],
                                lhsT=wb_all[:, t * 8:(t + 1) * 8],
                                rhs=eb[:, q * 512:q * 512 + qw],
                                start=(t == 0), stop=(t == NT - 1))
                            nc.tensor.matmul(
                                accf[q][:, 0:qw],
                                lhsT=wf_all[:, t * 4:(t + 1) * 4],
                                rhs=ef[:, q * 512:q * 512 + qw],
                                start=(t == 0), stop=(t == NT - 1))
                    for q in range(nq):
                        j0 = jc * b_w + q * 512
                        qw = min(512, b_w - q * 512)
                        ndsb = bep.tile([36, 512], F32, tag="ndsb",
                                        name="ndsb")
                        nc.scalar.copy(ndsb[0:8, 0:qw], accb[q][:, 0:qw])
                        nc.scalar.copy(ndsb[32:36, 0:qw], accf[q][:, 0:qw])
                        nc.sync.dma_start(nd_in[0:8, j0:j0 + qw],
                                          ndsb[0:8, 0:qw])
                        nc.sync.dma_start(nd_in[8:12, j0:j0 + qw],
                                          ndsb[32:36, 0:qw])
            nc.gpsimd.collective_compute(
                "AllReduce", ALU.add, replica_groups=rg,
                ins=[nd_in.opt()], outs=[nd_out.opt()])

            # ---------------- finalize (redundant on every core) ------------
            lp = L // 128  # plane free width
            with (
                tc.tile_pool(name="fin", bufs=1) as fp,
                tc.tile_pool(name="fin_ps", bufs=1, space="PSUM") as fps,
            ):
                def plane_from(dram_row, tg):
                    tl = fp.tile([128, lp], F32, tag=tg, name=tg)
                    nc.sync.dma_start(
                        tl[:], dram_row.rearrange("(p q) -> p q", p=128))
                    return tl

                nd = [plane_from(nd_out[k, :], f"nd{k}") for k in range(12)]
                rgbp = [plane_from(rgbpd[k, :], f"rgb{k}") for k in range(3)]
                keepf = plane_from(keepd[0, :], "keepf")
                predf = plane_from(predd[0, :], "predf")
                ktgt = plane_from(ktgtd[0, :], "ktgt")

                num, den = nd[0:3], nd[3]
                s0, cnt0 = nd[4:7], nd[7]
                sf, cntf = nd[8:11], nd[11]

                _cnt = [0]

                def newt():
                    _cnt[0] += 1
                    return fp.tile([128, lp], F32, tag=f"fin{_cnt[0]}",
                                   name=f"fin{_cnt[0]}")

                dsafe = newt()
                nc.vector.tensor_scalar(dsafe[:], den[:], 0.0, None,
                                        op0=ALU.is_equal)
                nc.vector.tensor_tensor(dsafe[:], dsafe[:], den[:], op=ALU.add)
                rden = newt()
                nc.vector.reciprocal(rden[:], dsafe[:])
                c0safe = newt()
                nc.vector.tensor_scalar(c0safe[:], cnt0[:], 0.0, None,
                                        op0=ALU.is_equal)
                nc.vector.tensor_tensor(c0safe[:], c0safe[:], cnt0[:],
                                        op=ALU.add)
                rcnt0 = newt()
                nc.vector.reciprocal(rcnt0[:], c0safe[:])
                rcntf = newt()
                nc.vector.reciprocal(rcntf[:], cntf[:])

                mden = fp.tile([128, lp], mybir.dt.int32, tag="mden",
                               name="mden")
                nc.vector.tensor_scalar(mden[:], den[:], 0.0, None,
                                        op0=ALU.not_equal)
                mz = fp.tile([128, lp], mybir.dt.int32, tag="mz", name="mz")
                nc.vector.tensor_scalar(mz[:], cnt0[:], 0.0, None,
                                        op0=ALU.is_gt)

                acc = newt()
                nc.vector.memset(acc[:], 0.0)
                for c in range(3):
                    rec = newt()
                    nc.vector.tensor_tensor(rec[:], sf[c][:], rcntf[:],
                                            op=ALU.mult)
                    tmp = newt()
                    nc.vector.tensor_tensor(tmp[:], num[c][:], rden[:],
                                            op=ALU.mult)
                    nc.vector.copy_predicated(rec[:], mden[:], tmp[:])
                    nc.vector.tensor_tensor(tmp[:], s0[c][:], rcnt0[:],
                                            op=ALU.mult)
                    nc.vector.copy_predicated(rec[:], mz[:], tmp[:])
                    diff = newt()
                    nc.vector.tensor_tensor(diff[:], rgbp[c][:], rec[:],
                                            op=ALU.subtract)
                    ad = newt()
                    nc.scalar.activation(ad[:], diff[:], ACTF.Abs)
                    nc.vector.tensor_tensor(acc[:], acc[:], ad[:], op=ALU.add)
                nc.vector.tensor_tensor(acc[:], acc[:], keepf[:], op=ALU.mult)

                # BCE: relu(p) - p*t + softplus(-|p|)
                bce = newt()
                nc.scalar.activation(bce[:], predf[:], ACTF.Relu)
                pt = newt()
                nc.vector.tensor_tensor(pt[:], predf[:], ktgt[:], op=ALU.mult)
                nc.vector.tensor_tensor(bce[:], bce[:], pt[:], op=ALU.subtract)
                ap_ = newt()
                nc.scalar.activation(ap_[:], predf[:], ACTF.Abs)
                en = newt()
                nc.scalar.activation(en[:], ap_[:], ACTF.Exp, scale=-1.0)
                sp = newt()
                nc.scalar.activation(sp[:], en[:], ACTF.Ln, bias=1.0)
                nc.vector.tensor_tensor(bce[:], bce[:], sp[:], op=ALU.add)

                rows2 = fp.tile([128, 2], F32, tag="rows2")
                nc.vector.tensor_reduce(rows2[:, 0:1], bce[:], axis=AX.X,
                                        op=ALU.add)
                nc.vector.tensor_reduce(rows2[:, 1:2], acc[:], axis=AX.X,
                                        op=ALU.add)
                onescol = fp.tile([128, 1], F32, tag="onescol")
                nc.vector.memset(onescol[:], 1.0)
                pstot = fps.tile([1, 2], F32, tag="pstot")
                nc.tensor.matmul(pstot[:], lhsT=onescol[:], rhs=rows2[:],
                                 start=True, stop=True)
                outsb = fp.tile([1, 2], F32, tag="outsb")
                nc.scalar.copy(outsb[:], pstot[:])
                nc.sync.dma_start(outd[:, :], outsb[:])

    nc.compile()
    return nc


def _host_prep(pred_F, cand_xyz, cand_rgb, tgt_xyz, tgt_rgb, keep_target,
               points_num):
    nsh = N // NCORES
    npad = NT * 128
    pred = np.ascontiguousarray(np.asarray(pred_F, np.float32))
    cxyz = np.ascontiguousarray(np.asarray(cand_xyz, np.float32))
    crgb = np.ascontiguousarray(np.asarray(cand_rgb, np.float32))
    txyz = np.ascontiguousarray(np.asarray(tgt_xyz, np.float32))
    trgb_np = np.ascontiguousarray(np.asarray(tgt_rgb, np.float32))
    ktgt = np.asarray(keep_target).astype(np.float32)

    # keep mask (exact reference semantics, f32)
    p8 = pred.reshape(-1, 8)
    rows = np.arange(p8.shape[0])
    ilm = np.zeros(p8.shape, dtype=bool)
    ilm[rows, np.argmax(p8, axis=1)] = True
    ilm = ilm.reshape(-1)
    k = L - int(points_num)
    vals = np.where(ilm, np.inf, pred)
    thr = np.sort(vals)[k - 1]
    keep = (pred > thr) | ilm

    b2 = np.sum(cxyz * cxyz, axis=1, dtype=np.float32).astype(np.float32)
    b2m = np.where(keep, b2, BIG).astype(np.float32)
    ones = np.ones(L, np.float32)
    c5r = np.ascontiguousarray(
        np.stack([cxyz[:, 0], cxyz[:, 1], cxyz[:, 2], ones, b2]))
    c5m = np.ascontiguousarray(
        np.stack([cxyz[:, 0], cxyz[:, 1], cxyz[:, 2], ones, b2m]))

    a2 = np.sum(txyz * txyz, axis=1, dtype=np.float32).astype(np.float32)

    t5_cores, trgb_cores = [], []
    for c in range(NCORES):
        sl = slice(c * nsh, (c + 1) * nsh)
        t5 = np.zeros((5, npad), np.float32)
        t5[3, :] = BIG     # pad rows: s = 1e30 everywhere
        t5[4, :] = 1.0
        t5[0, :nsh] = -2.0 * txyz[sl, 0]
        t5[1, :nsh] = -2.0 * txyz[sl, 1]
        t5[2, :nsh] = -2.0 * txyz[sl, 2]
        t5[3, :nsh] = a2[sl]
        tr = np.zeros((npad, 3), np.float32)
        tr[:nsh] = trgb_np[sl]
        # [p, t*3+c] layout: target i_local = t*128 + p
        trc = tr.reshape(NT, 128, 3).transpose(1, 0, 2).reshape(128, NT * 3)
        t5_cores.append(np.ascontiguousarray(t5))
        trgb_cores.append(np.ascontiguousarray(trc))

    rgbp = np.ascontiguousarray((crgb * np.float32(255.0)).T.astype(np.float32))
    keepf = keep.astype(np.float32).reshape(1, L)
    eye = np.eye(128, dtype=np.float32)

    common = dict(c5r=c5r, c5m=c5m, rgbp=rgbp, keepf=keepf,
                  predf=pred.reshape(1, L), ktgt=ktgt.reshape(1, L),
                  eye128=eye)
    in_maps = [dict(common, t5=t5_cores[c], trgb=trgb_cores[c])
               for c in range(NCORES)]
    return in_maps


_CACHE = {}


def kernel(pred_F, cand_xyz, cand_rgb, tgt_xyz, tgt_rgb, keep_target,
           points_num=8192, **_ignored):
    in_maps = _host_prep(pred_F, cand_xyz, cand_rgb, tgt_xyz, tgt_rgb,
                         keep_target, points_num)
    if "nc" not in _CACHE:
        _CACHE["nc"] = _build_nc()
    res = run_bass_kernel_spmd(_CACHE["nc"], in_maps,
                               core_ids=list(range(NCORES)))
    return np.asarray(res.results[0]["out"], np.float32).reshape(2)


if __name__ == "__main__":
    import reference as R
    inputs = R.setup_inputs()
    inputs = {kk: np.asarray(vv) if not np.isscalar(vv) else vv
              for kk, vv in inputs.items()}
    out = kernel(**inputs)
    print("kernel out:", out)
